# revision 2
# baseline (speedup 1.0000x reference)
"""Trainium2 Bass kernel for nn_Block_17978733101066 — fp8 DoubleRow version.

ConvNeXt-style block: channels-first LayerNorm -> NNMF conv (25 multiplicative
updates with grouped 3x3 convs) residual branch, then channels-last LayerNorm +
MLP residual branch.  Input x: (8, 96, 56, 56) f32, one sample per NeuronCore.

Key ideas vs the bf16 baseline:
- Grouped 3x3 convs run as fp8e4m3 DoubleRow matmuls (0.5 cycles/row): the
  padded image lives on a flat 64-column grid so vertically-adjacent offsets
  sit exactly 64 bytes apart (DoubleRow pair strides must be 16B-aligned,
  hw-validated); 9 offsets = 3 true pairs + 3 padded pairs = 6 matmuls/conv,
  3x fewer PE streaming cycles than the bf16 block-diag approach.  Weights
  are scaled by S=128 into fp8's normal range.
- Conv rhs views are contiguous flat spans; pad/dead columns carry garbage
  that stays exactly zero through the h-update chain (h_pad=0 by induction).
- A 97th all-ones partition row in hpad plus an eps slot in the pair-0
  weights folds "+eps" of the back-projection into the conv matmul for free.
- ratio = xin/(recon+eps) in the log domain: ACT Ln(psumA/S) -> DVE bf16
  subtract -> ACT Exp -> fp8 ratio, putting 2 of the 5 big elementwise ops
  per chunk on the otherwise-idle ACT engine.
- h-normalization: one (97,96) all-ones matmul broadcasts per-pixel channel
  sums (+1e-12 via its eps row against ht's ones-row) to all partitions; the
  S scale cancels between ht_s = h*psumB (no descale) and its normalization.
- 16-row slots (1024 flat px); 8-row sub-chunks = 512 flat px = one psum
  bank = the DoubleRow moving limit.  A-conv weights are reused across slot
  pairs (pair-major over 4 sub-chunks) to keep LDWEIGHTS off the critical
  path.  Final iteration writes h to a bf16 tile for output precision.
"""

import numpy as np

C = 96
H = W = 56
NPIX = H * W          # 3136
WP = 64               # padded row width (16B-aligned pair strides)
NROWS = 58
PADIMG = NROWS * WP   # 3712
G, CG = 4, 24
NIT = 25
EPS = 1e-12
HID = 384
S = 128.0             # fp8 weight scale
NF = 8 * WP           # 512 flat columns per 8-row chunk
CHUNKS = [(8 * k, 8) for k in range(7)]           # (y0, interior rows)
NCH = len(CHUNKS)
# 6 matmul slots per conv: 3 vertical pairs (delta 64) + 3 padded singles
MMS = [(0, 3), (1, 4), (2, 5), (6, None), (7, None), (8, None)]

TRACE = False
LAST_RESULT = None
_CACHED_NC = None


def _pair_view(base_ap, delta):
    """Insert a DoubleRow k-tile dim (stride=delta, count=2) after the
    partition dim of an AP."""
    from concourse.ap import AP
    ap_list = [list(p) for p in base_ap.ap]
    new_ap = [ap_list[0], [delta, 2]] + ap_list[1:]
    return AP(base_ap.tensor, base_ap.offset, new_ap)


def _build_weights(w_nnmf, e_slot):
    """fp8 DoubleRow pair weights for both convs + iteration-0 log-recon."""
    import ml_dtypes
    f8 = ml_dtypes.float8_e4m3
    w = np.abs(np.asarray(w_nnmf, np.float64))
    w = w / (w.sum(axis=(1, 2, 3), keepdims=True) + EPS)  # (96, 24, 3, 3)
    Wc = np.zeros((9, C, C), np.float64)  # [k, i, o]
    Wr = np.zeros((9, C, C), np.float64)  # [k, o, i]
    for dy in range(3):
        for dx in range(3):
            k = dy * 3 + dx
            blkc = w[:, :, dy, dx]          # (96 out, 24 in_local)
            blkr = w[:, :, 2 - dy, 2 - dx]
            for g in range(G):
                rows = slice(g * CG, (g + 1) * CG)
                Wc[k, rows, rows] = blkc[rows, :].T
                Wr[k, rows, rows] = blkr[rows, :]

    def q8(a):
        return np.asarray(a, np.float32).astype(f8).astype(np.float64)

    def packs(Wk):
        out = np.zeros((97, 6, 2, C), np.float64)
        for j, (ka, kb) in enumerate(MMS):
            out[0:C, j, 0, :] = q8(Wk[ka] * S)
            if kb is not None:
                out[0:C, j, 1, :] = q8(Wk[kb] * S)
        return out

    wrp = packs(Wr)
    wrp[C, 0, 0, :] = e_slot          # recon eps rides matmul-slot 0, tile 0
    wcp = packs(Wc)
    # iteration-0 back-projection from the constant h0=1/C fill using the
    # quantized device weights; ship ln(recon0+eps_eff) on the 64-wide grid.
    kmap = {}
    for j, (ka, kb) in enumerate(MMS):
        kmap[ka] = (j, 0)
        if kb is not None:
            kmap[kb] = (j, 1)
    hpad0 = np.zeros((C, NROWS, NROWS))
    hpad0[:, 1:1 + H, 1:1 + W] = 1.0 / C
    recon0 = np.zeros((C, H * W))
    for k in range(9):
        dy, dx = k // 3, k % 3
        j, t = kmap[k]
        wk = wrp[0:C, j, t, :] / S      # [i, o]
        view = hpad0[:, dy:dy + H, dx:dx + W].reshape(C, H * W)
        recon0 += wk.T @ view
    rec0 = np.zeros((C, NROWS, WP))
    rec0[:, 1:1 + H, 1:1 + W] = 1.0 / (
        S * (recon0.reshape(C, H, W) + e_slot / S))
    return (np.ascontiguousarray(wrp.reshape(97, 12 * C), np.float32),
            np.ascontiguousarray(wcp.reshape(97, 12 * C), np.float32),
            np.ascontiguousarray(rec0.reshape(C, PADIMG), np.float32))


def _build_bass(nit=NIT, gelu_mode="hw"):
    import concourse.bacc as bacc
    import concourse.mybir as mybir
    from concourse.tile import TileContext

    f32 = mybir.dt.float32
    bf16 = mybir.dt.bfloat16
    fp8 = mybir.dt.float8e4
    AF = mybir.ActivationFunctionType
    OP = mybir.AluOpType
    DR = mybir.MatmulPerfMode.DoubleRow

    nc = bacc.Bacc(None, target_bir_lowering=False)

    x_d = nc.declare_dram_parameter("x", [C, NPIX], f32, isOutput=False)
    rec0_d = nc.declare_dram_parameter("rec0", [C, PADIMG], bf16,
                                        isOutput=False)
    wrp_d = nc.declare_dram_parameter("wrp", [97, 12 * C], fp8, isOutput=False)
    wcp_d = nc.declare_dram_parameter("wcp", [97, 12 * C], fp8, isOutput=False)
    w1_d = nc.declare_dram_parameter("w1T", [C, HID], bf16, isOutput=False)
    b1_d = nc.declare_dram_parameter("b1", [HID, 1], f32, isOutput=False)
    w2_d = nc.declare_dram_parameter("w2T", [HID, C], bf16, isOutput=False)
    b2_d = nc.declare_dram_parameter("b2", [C, 1], f32, isOutput=False)
    ln1w_d = nc.declare_dram_parameter("ln1w", [C, 1], f32, isOutput=False)
    ln1b_d = nc.declare_dram_parameter("ln1b", [C, 1], f32, isOutput=False)
    out_d = nc.declare_dram_parameter("out", [C, NPIX], f32, isOutput=True)

    with TileContext(nc) as tc:
        with (
            tc.tile_pool(name="persist", bufs=1) as pp,
            tc.tile_pool(name="work", bufs=2) as wp,
            tc.tile_pool(name="psum", bufs=8, space="PSUM") as psp,
        ):
            # ---- persistent tiles ----
            xs = pp.tile([C, NPIX], f32, tag="xs")
            x2s = pp.tile([C, NPIX], f32, tag="x2s")
            xin_s = pp.tile([C, PADIMG], bf16, tag="xin_s")
            rec0 = pp.tile([C, PADIMG], bf16, tag="rec0")
            hbf = pp.tile([C, PADIMG], bf16, tag="hbf")
            # padded images with a 1-element guard at both ends
            hpad = pp.tile([97, PADIMG + 8], fp8, tag="hpad")
            rpad = pp.tile([97, PADIMG + 8], fp8, tag="rpad")
            wrp = pp.tile([97, 12 * C], fp8, tag="wrp")
            wcp = pp.tile([97, 12 * C], fp8, tag="wcp")
            ones97 = pp.tile([97, C], bf16, tag="ones97")
            w1s = pp.tile([C, HID], bf16, tag="w1s")
            w2s = [pp.tile([128, C], bf16, tag=f"w2s{k}", name=f"w2s{k}")
                   for k in range(3)]
            b1s = pp.tile([128, 3], f32, tag="b1s")
            b2s = pp.tile([C, 1], f32, tag="b2s")
            ln1w = pp.tile([C, 1], f32, tag="ln1w")
            ln1b = pp.tile([C, 1], f32, tag="ln1b")
            ones_col = pp.tile([C, 1], bf16, tag="ones_col")
            ones_row = pp.tile([1, C], bf16, tag="ones_row")
            ones_rowS = pp.tile([1, C], bf16, tag="ones_rowS")
            eps6 = pp.tile([1, 1], f32, tag="eps6")
            eps5 = pp.tile([1, 1], f32, tag="eps5")
            eps12 = pp.tile([1, 1], f32, tag="eps12")
            hts = [pp.tile([97, NF], bf16, tag=f"ht{i}", name=f"ht{i}")
                   for i in range(3)]

            # ---- input DMA ----
            nc.sync.dma_start(wrp[:], wrp_d[:])
            nc.sync.dma_start(wcp[:], wcp_d[:])
            nc.sync.dma_start(xs[:], x_d[:])
            nc.gpsimd.dma_start(ln1w[:], ln1w_d[:])
            nc.gpsimd.dma_start(ln1b[:], ln1b_d[:])
            nc.gpsimd.dma_start(rec0[:], rec0_d[:])
            nc.gpsimd.dma_start(w1s[:], w1_d[:])
            for k in range(3):
                nc.gpsimd.dma_start(w2s[k][:], w2_d[k * 128:(k + 1) * 128, :])
            nc.gpsimd.dma_start(b1s[:],
                                b1_d[:].rearrange("(k p) one -> p (k one)", p=128))
            nc.gpsimd.dma_start(b2s[:], b2_d[:])

            nc.vector.memset(ones_col[:], 1.0)
            nc.vector.memset(ones_row[:], 1.0)
            nc.vector.memset(ones_rowS[:], S)
            nc.vector.memset(eps6[:], 1e-6)
            nc.vector.memset(eps5[:], 1e-5)
            nc.vector.memset(eps12[:], 1e-12)
            nc.vector.memset(ones97[0:C, :], 1.0)
            nc.vector.memset(ones97[C:97, :], S * EPS)   # norm-sum eps row
            for t in hts:
                nc.vector.memset(t[C:97, :], 1.0)        # ht ones row

            nc.vector.memset(hpad[0:C, :], 0.0)
            nc.vector.memset(rpad[0:C, :], 0.0)
            nc.vector.memset(hpad[C:97, :], 1.0)
            nc.vector.memset(rpad[C:97, :], 1.0)
            nc.vector.memset(
                hpad[0:C, 1:1 + PADIMG].rearrange(
                    "p (h w) -> p h w", w=WP)[0:C, 1:1 + H, 1:1 + W],
                1.0 / C)
            nc.vector.memset(xin_s[:], 0.0)
            nc.vector.memset(hbf[:], 0.0)
            nc.vector.memset(
                hbf[:].rearrange("p (h w) -> p h w",
                                 w=WP)[0:C, 1:1 + H, 1:1 + W], 1.0 / C)

            wrp4 = wrp[:].rearrange("p (j two m) -> p j two m", j=6, two=2)
            wcp4 = wcp[:].rearrange("p (j two m) -> p j two m", j=6, two=2)

            def psum_tile(name):
                # (128, 512) f32 = exactly 1 bank
                return psp.tile([128, 512], f32, tag="cv", name=name)

            def span(t, y0, nr, p=C, guard=1):
                st = guard + (1 + y0) * WP
                return t[0:p, st:st + nr * WP]

            def conv_mms(wp4, src, y0, ps):
                """DoubleRow conv matmuls for one 8-row chunk."""
                for j, (ka, kb) in enumerate(MMS):
                    dya, dxa = ka // 3, ka % 3
                    d = 64 if kb is not None else 0
                    base = (y0 + dya) * WP + dxa
                    view = src[0:97, base:base + NF]
                    nc.tensor.matmul(
                        ps[0:C, 0:NF],
                        wp4[0:97, j, :, :], _pair_view(view, d),
                        perf_mode=DR, start=(j == 0), stop=(j == 5),
                        skip_group_check=True)

            # ================= NNMF slot phases =================
            def phase_a(it, c):
                """A-conv -> recip -> ratio (GpSimd) -> rpad for chunk c."""
                y0, nr = CHUNKS[c]
                if it == 0:
                    nc.gpsimd.tensor_tensor(
                        span(rpad, y0, nr), span(xin_s, y0, nr, guard=0),
                        span(rec0, y0, nr, guard=0), OP.mult)
                    return
                ps = psum_tile("psA")
                conv_mms(wrp4, hpad, y0, ps)
                rec = wp.tile([C, NF], f32, tag="rec", bufs=4)
                nc.vector.reciprocal_approx_fast(out=rec[:], in_=ps[0:C, 0:NF])
                nc.gpsimd.tensor_tensor(
                    span(rpad, y0, nr), span(xin_s, y0, nr, guard=0),
                    rec[:], OP.mult)

            def phase_b(it, c):
                """B-conv + ht_s = h * psumB for chunk c."""
                y0, nr = CHUNKS[c]
                ps = psum_tile("psB")
                conv_mms(wcp4, rpad, y0, ps)
                ht = hts[(it * NCH + c) % 3]
                nc.vector.tensor_tensor(ht[0:C, 0:NF],
                                        span(hbf, y0, nr, guard=0),
                                        ps[0:C, 0:NF], OP.mult)
                return ht

            def phase_n(it, c, ht):
                ps = psum_tile("psN")
                nc.tensor.matmul(ps[0:C, 0:NF], ones97[:], ht[0:97, 0:NF],
                                 start=True, stop=True)
                sN = wp.tile([C, NF], f32, tag="sN", bufs=3)
                nc.vector.reciprocal_approx_fast(out=sN[:], in_=ps[0:C, 0:NF])
                return sN

            def phase_b3(it, c, ht, sN):
                y0, nr = CHUNKS[c]
                nc.gpsimd.tensor_tensor(span(hbf, y0, nr, guard=0),
                                        ht[0:C, 0:NF], sN[:], OP.mult)
                if it < nit - 1:
                    nc.vector.tensor_tensor(span(hpad, y0, nr),
                                            ht[0:C, 0:NF], sN[:], OP.mult)

            # ================= LN1 (448px compact chunks -> lxin) ==========
            CW1 = 448

            def rowsum(src_ap, name="csum"):
                s = psum_tile(name)
                nc.tensor.matmul(s[0:1, 0:CW1], ones_col[:], src_ap)
                return s

            def bcast(row_ap, name="bc", scaled=False):
                b = psum_tile(name)
                nc.tensor.matmul(b[0:C, 0:CW1],
                                 ones_rowS[:] if scaled else ones_row[:],
                                 row_ap)
                return b

            def colsum96(src_ap):
                s = rowsum(src_ap)
                t = wp.tile([C + 1, CW1], f32, tag="cs_t", bufs=2)
                nc.scalar.activation(t[0:1, :], s[0:1, 0:CW1], AF.Identity,
                                     bias=eps12[:, 0:1])
                rsf = wp.tile([C + 1, CW1], f32, tag="cs_rf", bufs=2)
                nc.vector.reciprocal_approx_fast(out=rsf[0:1, :], in_=t[0:1, :])
                rs = wp.tile([C + 1, CW1], bf16, tag="cs_r", bufs=2)
                nc.vector.tensor_copy(rs[0:1, :], rsf[0:1, :])
                return rs

            def ln_stats(xc_f32, xc_bf16, eps_tile):
                sq = wp.tile([C, CW1], bf16, tag="ln_sq", bufs=2)
                nc.scalar.square(sq[:], xc_f32)
                s1 = rowsum(xc_bf16)
                s2 = rowsum(sq[:])
                u = wp.tile([C + 1, CW1], bf16, tag="ln_u", bufs=2)
                with nc.allow_low_precision(reason="bf16 broadcast operand"):
                    nc.vector.tensor_scalar_mul(u[0:1, :], s1[0:1, 0:CW1],
                                                1.0 / C)
                u2 = wp.tile([C + 1, CW1], f32, tag="ln_u2", bufs=2)
                nc.scalar.square(u2[0:1, :], u[0:1, :])
                var = wp.tile([C + 1, CW1], f32, tag="ln_var", bufs=2)
                nc.vector.scalar_tensor_tensor(
                    var[0:1, :], s2[0:1, 0:CW1], 1.0 / C, u2[0:1, :],
                    OP.mult, OP.subtract)
                sd = wp.tile([C + 1, CW1], f32, tag="ln_sd", bufs=2)
                nc.scalar.activation(sd[0:1, :], var[0:1, :], AF.Sqrt,
                                     bias=eps_tile[:, 0:1])
                isdf = wp.tile([C + 1, CW1], f32, tag="ln_isdf", bufs=2)
                nc.vector.reciprocal_approx_fast(out=isdf[0:1, :],
                                                 in_=sd[0:1, :])
                isd = wp.tile([C + 1, CW1], bf16, tag="ln_isd", bufs=2)
                nc.scalar.copy(isd[0:1, :], isdf[0:1, :])
                return u, isd

            def ln1_chunk(c):
                sl = slice(c * CW1, (c + 1) * CW1)
                xc = xs[:, sl]
                xbc = wp.tile([C, CW1], bf16, tag="x2b", bufs=2)
                nc.scalar.copy(xbc[:], xc)
                u, isd = ln_stats(xc, xbc[:], eps6)
                ub = bcast(u[0:1, :])
                ib = bcast(isd[0:1, :])
                xm = wp.tile([C, CW1], f32, tag="ln_xm", bufs=2)
                nc.vector.tensor_tensor(xm[:], xc, ub[0:C, 0:CW1], OP.subtract)
                xn = wp.tile([C, CW1], f32, tag="ln_xn", bufs=2)
                nc.vector.tensor_tensor(xn[:], xm[:], ib[0:C, 0:CW1], OP.mult)
                rl = wp.tile([C, CW1], bf16, tag="ln_rl", bufs=2)
                nc.scalar.activation(rl[:], xn[:], AF.Relu,
                                     bias=ln1b[:, 0:1], scale=ln1w[:, 0:1])
                rs = colsum96(rl[:])
                sb = bcast(rs[0:1, :], scaled=True)
                lx3 = xin_s[:].rearrange("p (h w) -> p h w", w=WP)
                nc.vector.tensor_tensor(
                    lx3[0:C, 1 + 8 * c:1 + 8 * c + 8, 1:1 + W],
                    rl[:].rearrange("p (h w) -> p h w", w=W),
                    sb[0:C, 0:CW1].rearrange("p (h w) -> p h w", w=W), OP.mult)

            # ================= MLP epilogue (448px compact chunks) ==========
            hf3 = hbf[:].rearrange("p (h w) -> p h w", w=WP)

            def mlp_p1(c):
                sl = slice(c * CW1, (c + 1) * CW1)
                nc.gpsimd.tensor_tensor(
                    x2s[:, sl].rearrange("p (h w) -> p h w", w=W),
                    xs[:, sl].rearrange("p (h w) -> p h w", w=W),
                    hf3[0:C, 1 + 8 * c:1 + 8 * c + 8, 1:1 + W], OP.add)
                xc = x2s[:, sl]
                x2b = wp.tile([C, CW1], bf16, tag="x2b", bufs=2)
                nc.scalar.copy(x2b[:], xc)
                return ln_stats(xc, x2b[:], eps5)

            def mlp_p2(c, st):
                u, isd = st
                sl = slice(c * CW1, (c + 1) * CW1)
                xc = x2s[:, sl]
                ub = bcast(u[0:1, :])
                ib = bcast(isd[0:1, :])
                xm = wp.tile([C, CW1], f32, tag="ln_xm", bufs=2)
                nc.vector.tensor_tensor(xm[:], xc, ub[0:C, 0:CW1], OP.subtract)
                xn = wp.tile([C, CW1], bf16, tag="ln_xw", bufs=8)
                nc.vector.tensor_tensor(xn[:], xm[:], ib[0:C, 0:CW1], OP.mult)
                return xn

            def mlp_p3(c, xn):
                ys = []
                for j in range(3):
                    p1 = psum_tile("p1")
                    nc.tensor.matmul(p1[0:128, 0:CW1],
                                     w1s[:, j * 128:(j + 1) * 128], xn[:])
                    y1 = wp.tile([128, CW1], bf16, tag=f"mlp_y{j}",
                                 name=f"mlp_y{j}", bufs=2)
                    if gelu_mode == "hw":
                        nc.scalar.activation(y1[:], p1[0:128, 0:CW1], AF.Gelu,
                                             bias=b1s[:, j:j + 1])
                    else:
                        pre = wp.tile([128, CW1], f32, tag=f"mlp_p{j}",
                                      name=f"mlp_p{j}", bufs=2)
                        nc.scalar.activation(pre[:], p1[0:128, 0:CW1],
                                             AF.Identity, bias=b1s[:, j:j + 1])
                        sg = wp.tile([128, CW1], f32, tag=f"mlp_s{j}",
                                     name=f"mlp_s{j}", bufs=2)
                        nc.scalar.activation(sg[:], pre[:], AF.Sigmoid,
                                             scale=1.702)
                        nc.vector.tensor_tensor(y1[:], pre[:], sg[:], OP.mult)
                    ys.append(y1)
                return ys

            def mlp_p4(c, ys):
                sl = slice(c * CW1, (c + 1) * CW1)
                p2 = psum_tile("p2")
                for k in range(3):
                    nc.tensor.matmul(p2[0:C, 0:CW1], w2s[k][:], ys[k][:],
                                     start=(k == 0), stop=(k == 2))
                oc = wp.tile([C, CW1], f32, tag="oc", bufs=2)
                nc.vector.scalar_tensor_tensor(
                    oc[:], p2[0:C, 0:CW1], b2s[:, 0:1], x2s[:, sl],
                    OP.add, OP.add)
                nc.sync.dma_start(out_d[:, sl], oc[:])

            # ================= the global pipeline =================
            # slot s covers chunk s%7 of iteration s//7.  Lags match the
            # proven baseline: B at 2, norm at 3, b3 at 4.
            total = nit * NCH
            ht_live = {}
            sn_live = {}
            for s in range(0, total + 4):
                if s < total:
                    it0, cc0 = divmod(s, NCH)
                    if it0 == 0 and cc0 < 7:
                        ln1_chunk(cc0)
                    phase_a(it0, cc0)
                c1 = s - 2
                if 0 <= c1 < total:
                    it1, cc1 = divmod(c1, NCH)
                    ht_live[c1] = phase_b(it1, cc1)
                c2 = s - 3
                if 0 <= c2 < total:
                    it2, cc2 = divmod(c2, NCH)
                    sn_live[c2] = phase_n(it2, cc2, ht_live[c2])
                c3 = s - 4
                if 0 <= c3 < total:
                    it3, cc3 = divmod(c3, NCH)
                    phase_b3(it3, cc3, ht_live.pop(c3), sn_live.pop(c3))

            # MLP epilogue pipeline
            sts = {}
            xns = {}
            yss = {}
            for s in range(0, 7 + 3):
                if s < 7:
                    sts[s] = mlp_p1(s)
                m2 = s - 1
                if 0 <= m2 < 7:
                    xns[m2] = mlp_p2(m2, sts.pop(m2))
                m3 = s - 2
                if 0 <= m3 < 7:
                    yss[m3] = mlp_p3(m3, xns.pop(m3))
                m4 = s - 3
                if 0 <= m4 < 7:
                    mlp_p4(m4, yss.pop(m4))

    return nc


def _prepare_maps(x, ln1_w, ln1_b, w_nnmf, ln2_w, ln2_b, w1, b1, w2, b2):
    import ml_dtypes
    bf16 = ml_dtypes.bfloat16
    f8 = ml_dtypes.float8_e4m3
    # eps_eff sized so ratio = xin/(recon+eps_eff) <= ~212 < fp8 max 240:
    # the Exp->fp8 cast produces inf (not saturation) on overflow.
    xf = np.asarray(x, np.float64)
    u = xf.mean(axis=1, keepdims=True)
    v = ((xf - u) ** 2).mean(axis=1, keepdims=True)
    xnorm = ((xf - u) / np.sqrt(v + 1e-6)
             * np.asarray(ln1_w, np.float64)[None, :, None, None]
             + np.asarray(ln1_b, np.float64)[None, :, None, None])
    xr = np.maximum(xnorm, 0.0)
    xin_max = float((xr / (xr.sum(axis=1, keepdims=True) + EPS)).max())
    e_slot = S * max(xin_max, 1e-3) / 200.0
    wrp, wcp, rec0 = _build_weights(w_nnmf, e_slot)
    f = lambda a: np.ascontiguousarray(np.asarray(a, np.float32))
    fb = lambda a: np.ascontiguousarray(np.asarray(a, np.float32).astype(bf16))
    f8c = lambda a: np.ascontiguousarray(np.asarray(a, np.float32).astype(f8))
    w1_64 = np.asarray(w1, np.float64)
    w1f = w1_64 * np.asarray(ln2_w, np.float64)[:, None]
    b1f = np.asarray(b1, np.float64) + np.asarray(ln2_b, np.float64) @ w1_64
    shared = {
        "rec0": fb(rec0),
        "wrp": f8c(wrp),
        "wcp": f8c(wcp),
        "w1T": fb(w1f),
        "b1": f(b1f).reshape(HID, 1),
        "w2T": fb(w2),
        "b2": f(b2).reshape(C, 1),
        "ln1w": f(ln1_w).reshape(C, 1),
        "ln1b": f(ln1_b).reshape(C, 1),
    }
    xs = np.asarray(x)
    return [dict(shared, x=f(xs[i]).reshape(C, NPIX))
            for i in range(xs.shape[0])]


def kernel(x, ln1_w, ln1_b, w_nnmf, ln2_w, ln2_b, w1, b1, w2, b2):
    global _CACHED_NC, LAST_RESULT
    from concourse.bass_utils import run_bass_kernel_spmd

    if _CACHED_NC is None:
        nc = _build_bass()
        nc.finalize()
        _CACHED_NC = nc
    nc = _CACHED_NC
    in_maps = _prepare_maps(x, ln1_w, ln1_b, w_nnmf, ln2_w, ln2_b, w1, b1, w2, b2)
    res = run_bass_kernel_spmd(nc, in_maps, core_ids=list(range(8)), trace=TRACE)
    LAST_RESULT = res
    out = np.stack([res.results[i]["out"].reshape(C, H, W) for i in range(8)])
    return out.astype(np.float32)


# revision 3
# speedup vs baseline: 1.0211x; 1.0211x over previous
"""Trainium2 Bass kernel for nn_Block_17978733101066 — fp8 DoubleRow version.

ConvNeXt-style block: channels-first LayerNorm -> NNMF conv (25 multiplicative
updates with grouped 3x3 convs) residual branch, then channels-last LayerNorm +
MLP residual branch.  Input x: (8, 96, 56, 56) f32, one sample per NeuronCore.

Key ideas vs the bf16 baseline:
- Grouped 3x3 convs run as fp8e4m3 DoubleRow matmuls (0.5 cycles/row): the
  padded image lives on a flat 64-column grid so vertically-adjacent offsets
  sit exactly 64 bytes apart (DoubleRow pair strides must be 16B-aligned,
  hw-validated); 9 offsets = 3 true pairs + 3 padded pairs = 6 matmuls/conv,
  3x fewer PE streaming cycles than the bf16 block-diag approach.  Weights
  are scaled by S=128 into fp8's normal range.
- Conv rhs views are contiguous flat spans; pad/dead columns carry garbage
  that stays exactly zero through the h-update chain (h_pad=0 by induction).
- A 97th all-ones partition row in hpad plus an eps slot in the pair-0
  weights folds "+eps" of the back-projection into the conv matmul for free.
- ratio = xin/(recon+eps) in the log domain: ACT Ln(psumA/S) -> DVE bf16
  subtract -> ACT Exp -> fp8 ratio, putting 2 of the 5 big elementwise ops
  per chunk on the otherwise-idle ACT engine.
- h-normalization: one (97,96) all-ones matmul broadcasts per-pixel channel
  sums (+1e-12 via its eps row against ht's ones-row) to all partitions; the
  S scale cancels between ht_s = h*psumB (no descale) and its normalization.
- 16-row slots (1024 flat px); 8-row sub-chunks = 512 flat px = one psum
  bank = the DoubleRow moving limit.  A-conv weights are reused across slot
  pairs (pair-major over 4 sub-chunks) to keep LDWEIGHTS off the critical
  path.  Final iteration writes h to a bf16 tile for output precision.
"""

import numpy as np

C = 96
H = W = 56
NPIX = H * W          # 3136
WP = 64               # padded row width (16B-aligned pair strides)
NROWS = 58
PADIMG = NROWS * WP   # 3712
G, CG = 4, 24
NIT = 25
EPS = 1e-12
HID = 384
S = 128.0             # fp8 weight scale
NF = 8 * WP           # 512 flat columns per 8-row chunk
CHUNKS = [(8 * k, 8) for k in range(7)]           # (y0, interior rows)
NCH = len(CHUNKS)
# 6 matmul slots per conv: 3 vertical pairs (delta 64) + 3 padded singles
MMS = [(0, 3), (1, 4), (2, 5), (6, None), (7, None), (8, None)]

TRACE = False
LAST_RESULT = None
_CACHED_NC = None


def _pair_view(base_ap, delta):
    """Insert a DoubleRow k-tile dim (stride=delta, count=2) after the
    partition dim of an AP."""
    from concourse.ap import AP
    ap_list = [list(p) for p in base_ap.ap]
    new_ap = [ap_list[0], [delta, 2]] + ap_list[1:]
    return AP(base_ap.tensor, base_ap.offset, new_ap)


def _build_weights(w_nnmf, e_slot):
    """fp8 DoubleRow pair weights for both convs + iteration-0 log-recon."""
    import ml_dtypes
    f8 = ml_dtypes.float8_e4m3
    w = np.abs(np.asarray(w_nnmf, np.float64))
    w = w / (w.sum(axis=(1, 2, 3), keepdims=True) + EPS)  # (96, 24, 3, 3)
    Wc = np.zeros((9, C, C), np.float64)  # [k, i, o]
    Wr = np.zeros((9, C, C), np.float64)  # [k, o, i]
    for dy in range(3):
        for dx in range(3):
            k = dy * 3 + dx
            blkc = w[:, :, dy, dx]          # (96 out, 24 in_local)
            blkr = w[:, :, 2 - dy, 2 - dx]
            for g in range(G):
                rows = slice(g * CG, (g + 1) * CG)
                Wc[k, rows, rows] = blkc[rows, :].T
                Wr[k, rows, rows] = blkr[rows, :]

    def q8(a):
        return np.asarray(a, np.float32).astype(f8).astype(np.float64)

    def packs(Wk):
        out = np.zeros((97, 6, 2, C), np.float64)
        for j, (ka, kb) in enumerate(MMS):
            out[0:C, j, 0, :] = q8(Wk[ka] * S)
            if kb is not None:
                out[0:C, j, 1, :] = q8(Wk[kb] * S)
        return out

    wrp = packs(Wr)
    wrp[C, 0, 0, :] = e_slot          # recon eps rides matmul-slot 0, tile 0
    wcp = packs(Wc)
    # iteration-0 back-projection from the constant h0=1/C fill using the
    # quantized device weights; ship ln(recon0+eps_eff) on the 64-wide grid.
    kmap = {}
    for j, (ka, kb) in enumerate(MMS):
        kmap[ka] = (j, 0)
        if kb is not None:
            kmap[kb] = (j, 1)
    hpad0 = np.zeros((C, NROWS, NROWS))
    hpad0[:, 1:1 + H, 1:1 + W] = 1.0 / C
    recon0 = np.zeros((C, H * W))
    for k in range(9):
        dy, dx = k // 3, k % 3
        j, t = kmap[k]
        wk = wrp[0:C, j, t, :] / S      # [i, o]
        view = hpad0[:, dy:dy + H, dx:dx + W].reshape(C, H * W)
        recon0 += wk.T @ view
    rec0 = np.zeros((C, NROWS, WP))
    rec0[:, 1:1 + H, 1:1 + W] = 1.0 / (
        S * (recon0.reshape(C, H, W) + e_slot / S))
    return (np.ascontiguousarray(wrp.reshape(97, 12 * C), np.float32),
            np.ascontiguousarray(wcp.reshape(97, 12 * C), np.float32),
            np.ascontiguousarray(rec0.reshape(C, PADIMG), np.float32))


def _build_bass(nit=NIT, gelu_mode="hw"):
    import concourse.bacc as bacc
    import concourse.mybir as mybir
    from concourse.tile import TileContext

    f32 = mybir.dt.float32
    bf16 = mybir.dt.bfloat16
    fp8 = mybir.dt.float8e4
    AF = mybir.ActivationFunctionType
    OP = mybir.AluOpType
    DR = mybir.MatmulPerfMode.DoubleRow

    nc = bacc.Bacc(None, target_bir_lowering=False)

    x_d = nc.declare_dram_parameter("x", [C, NPIX], f32, isOutput=False)
    rec0_d = nc.declare_dram_parameter("rec0", [C, PADIMG], bf16,
                                        isOutput=False)
    wrp_d = nc.declare_dram_parameter("wrp", [97, 12 * C], fp8, isOutput=False)
    wcp_d = nc.declare_dram_parameter("wcp", [97, 12 * C], fp8, isOutput=False)
    w1_d = nc.declare_dram_parameter("w1T", [C, HID], bf16, isOutput=False)
    b1_d = nc.declare_dram_parameter("b1", [HID, 1], f32, isOutput=False)
    w2_d = nc.declare_dram_parameter("w2T", [HID, C], bf16, isOutput=False)
    b2_d = nc.declare_dram_parameter("b2", [C, 1], f32, isOutput=False)
    ln1w_d = nc.declare_dram_parameter("ln1w", [C, 1], f32, isOutput=False)
    ln1b_d = nc.declare_dram_parameter("ln1b", [C, 1], f32, isOutput=False)
    out_d = nc.declare_dram_parameter("out", [C, NPIX], f32, isOutput=True)

    with TileContext(nc) as tc:
        with (
            tc.tile_pool(name="persist", bufs=1) as pp,
            tc.tile_pool(name="work", bufs=2) as wp,
            tc.tile_pool(name="psum", bufs=8, space="PSUM") as psp,
        ):
            # ---- persistent tiles ----
            xs = pp.tile([C, NPIX], f32, tag="xs")
            x2s = pp.tile([C, NPIX], f32, tag="x2s")
            xin_s = pp.tile([C, PADIMG], bf16, tag="xin_s")
            rec0 = pp.tile([C, PADIMG], bf16, tag="rec0")
            hbf = pp.tile([C, PADIMG], bf16, tag="hbf")
            # padded images with a 1-element guard at both ends
            hpad = pp.tile([97, PADIMG + 8], fp8, tag="hpad")
            rpad = pp.tile([97, PADIMG + 8], fp8, tag="rpad")
            wrp = pp.tile([97, 12 * C], fp8, tag="wrp")
            wcp = pp.tile([97, 12 * C], fp8, tag="wcp")
            ones97 = pp.tile([97, C], bf16, tag="ones97")
            w1s = pp.tile([C, HID], bf16, tag="w1s")
            w2s = [pp.tile([128, C], bf16, tag=f"w2s{k}", name=f"w2s{k}")
                   for k in range(3)]
            b1s = pp.tile([128, 3], f32, tag="b1s")
            b2s = pp.tile([C, 1], f32, tag="b2s")
            ln1w = pp.tile([C, 1], f32, tag="ln1w")
            ln1b = pp.tile([C, 1], f32, tag="ln1b")
            ones_col = pp.tile([C, 1], bf16, tag="ones_col")
            onesC = pp.tile([C, C], bf16, tag="onesC")      # all 1/C
            ones96 = pp.tile([C, C], bf16, tag="ones96")    # all 1.0
            ones_row = pp.tile([1, C], bf16, tag="ones_row")
            ones_rowS = pp.tile([1, C], bf16, tag="ones_rowS")
            eps6 = pp.tile([C, 1], f32, tag="eps6")
            eps5 = pp.tile([C, 1], f32, tag="eps5")
            eps12 = pp.tile([C, 1], f32, tag="eps12")
            hts = [pp.tile([97, NF], bf16, tag=f"ht{i}", name=f"ht{i}")
                   for i in range(3)]

            # ---- input DMA ----
            nc.sync.dma_start(wrp[:], wrp_d[:])
            nc.sync.dma_start(wcp[:], wcp_d[:])
            nc.sync.dma_start(xs[:], x_d[:])
            nc.gpsimd.dma_start(ln1w[:], ln1w_d[:])
            nc.gpsimd.dma_start(ln1b[:], ln1b_d[:])
            nc.gpsimd.dma_start(rec0[:], rec0_d[:])
            nc.gpsimd.dma_start(w1s[:], w1_d[:])
            for k in range(3):
                nc.gpsimd.dma_start(w2s[k][:], w2_d[k * 128:(k + 1) * 128, :])
            nc.gpsimd.dma_start(b1s[:],
                                b1_d[:].rearrange("(k p) one -> p (k one)", p=128))
            nc.gpsimd.dma_start(b2s[:], b2_d[:])

            nc.vector.memset(ones_col[:], 1.0)
            nc.vector.memset(onesC[:], 1.0 / C)
            nc.vector.memset(ones96[:], 1.0)
            nc.vector.memset(ones_row[:], 1.0)
            nc.vector.memset(ones_rowS[:], S)
            nc.vector.memset(eps6[:], 1e-6)
            nc.vector.memset(eps5[:], 1e-5)
            nc.vector.memset(eps12[:], 1e-12)
            nc.vector.memset(ones97[0:C, :], 1.0)
            nc.vector.memset(ones97[C:97, :], S * EPS)   # norm-sum eps row
            for t in hts:
                nc.vector.memset(t[C:97, :], 1.0)        # ht ones row

            nc.vector.memset(hpad[0:C, :], 0.0)
            nc.vector.memset(rpad[0:C, :], 0.0)
            nc.vector.memset(hpad[C:97, :], 1.0)
            nc.vector.memset(rpad[C:97, :], 1.0)
            nc.vector.memset(
                hpad[0:C, 1:1 + PADIMG].rearrange(
                    "p (h w) -> p h w", w=WP)[0:C, 1:1 + H, 1:1 + W],
                1.0 / C)
            nc.vector.memset(xin_s[:], 0.0)
            nc.vector.memset(hbf[:], 0.0)
            nc.vector.memset(
                hbf[:].rearrange("p (h w) -> p h w",
                                 w=WP)[0:C, 1:1 + H, 1:1 + W], 1.0 / C)

            wrp4 = wrp[:].rearrange("p (j two m) -> p j two m", j=6, two=2)
            wcp4 = wcp[:].rearrange("p (j two m) -> p j two m", j=6, two=2)

            def psum_tile(name):
                # (128, 512) f32 = exactly 1 bank
                return psp.tile([128, 512], f32, tag="cv", name=name)

            def span(t, y0, nr, p=C, guard=1):
                st = guard + (1 + y0) * WP
                return t[0:p, st:st + nr * WP]

            def conv_mms(wp4, src, y0, ps):
                """DoubleRow conv matmuls for one 8-row chunk."""
                for j, (ka, kb) in enumerate(MMS):
                    dya, dxa = ka // 3, ka % 3
                    d = 64 if kb is not None else 0
                    base = (y0 + dya) * WP + dxa
                    view = src[0:97, base:base + NF]
                    nc.tensor.matmul(
                        ps[0:C, 0:NF],
                        wp4[0:97, j, :, :], _pair_view(view, d),
                        perf_mode=DR, start=(j == 0), stop=(j == 5),
                        skip_group_check=True)

            # ================= NNMF slot phases =================
            def phase_a(it, c):
                """A-conv -> recip -> ratio (GpSimd) -> rpad for chunk c."""
                y0, nr = CHUNKS[c]
                if it == 0:
                    nc.gpsimd.tensor_tensor(
                        span(rpad, y0, nr), span(xin_s, y0, nr, guard=0),
                        span(rec0, y0, nr, guard=0), OP.mult)
                    return
                ps = psum_tile("psA")
                conv_mms(wrp4, hpad, y0, ps)
                rec = wp.tile([C, NF], f32, tag="rec", bufs=4)
                nc.vector.reciprocal_approx_fast(out=rec[:], in_=ps[0:C, 0:NF])
                nc.gpsimd.tensor_tensor(
                    span(rpad, y0, nr), span(xin_s, y0, nr, guard=0),
                    rec[:], OP.mult)

            def phase_b(it, c):
                """B-conv + ht_s = h * psumB for chunk c."""
                y0, nr = CHUNKS[c]
                ps = psum_tile("psB")
                conv_mms(wcp4, rpad, y0, ps)
                ht = hts[(it * NCH + c) % 3]
                nc.vector.tensor_tensor(ht[0:C, 0:NF],
                                        span(hbf, y0, nr, guard=0),
                                        ps[0:C, 0:NF], OP.mult)
                return ht

            def phase_n(it, c, ht):
                ps = psum_tile("psN")
                nc.tensor.matmul(ps[0:C, 0:NF], ones97[:], ht[0:97, 0:NF],
                                 start=True, stop=True)
                sN = wp.tile([C, NF], f32, tag="sN", bufs=3)
                nc.vector.reciprocal_approx_fast(out=sN[:], in_=ps[0:C, 0:NF])
                return sN

            def phase_b3(it, c, ht, sN):
                y0, nr = CHUNKS[c]
                nc.gpsimd.tensor_tensor(span(hbf, y0, nr, guard=0),
                                        ht[0:C, 0:NF], sN[:], OP.mult)
                if it < nit - 1:
                    nc.vector.tensor_copy(span(hpad, y0, nr),
                                          span(hbf, y0, nr, guard=0))

            # ================= LN1 (448px compact chunks -> lxin) ==========
            CW1 = 448

            def rowsum(src_ap, name="csum"):
                s = psum_tile(name)
                nc.tensor.matmul(s[0:1, 0:CW1], ones_col[:], src_ap)
                return s

            def bcast(row_ap, name="bc", scaled=False):
                b = psum_tile(name)
                nc.tensor.matmul(b[0:C, 0:CW1],
                                 ones_rowS[:] if scaled else ones_row[:],
                                 row_ap)
                return b

            def colsum96(src_ap):
                # broadcast channel sums via all-ones matmul, +eps, recip
                s = psum_tile("csb")
                nc.tensor.matmul(s[0:C, 0:CW1], ones96[:], src_ap)
                t = wp.tile([C, CW1], f32, tag="cs_t", bufs=2)
                nc.scalar.activation(t[:], s[0:C, 0:CW1], AF.Identity,
                                     bias=eps12[:, 0:1])
                rsf = wp.tile([C, CW1], f32, tag="cs_rf", bufs=2)
                nc.vector.reciprocal_approx_fast(out=rsf[:], in_=t[:])
                return rsf

            def ln_stats(xc_f32, xc_bf16, eps_tile):
                # broadcast-form stats: u_b and isd_b are (96, CW1) tiles
                sq = wp.tile([C, CW1], bf16, tag="ln_sq", bufs=2)
                nc.scalar.square(sq[:], xc_f32)
                s1 = psum_tile("s1b")
                nc.tensor.matmul(s1[0:C, 0:CW1], onesC[:], xc_bf16)
                s2 = psum_tile("s2b")
                nc.tensor.matmul(s2[0:C, 0:CW1], onesC[:], sq[:])
                u2 = wp.tile([C, CW1], f32, tag="ln_u2", bufs=2)
                nc.scalar.square(u2[:], s1[0:C, 0:CW1])
                var = wp.tile([C, CW1], f32, tag="ln_var", bufs=2)
                nc.vector.tensor_tensor(var[:], s2[0:C, 0:CW1], u2[:],
                                        OP.subtract)
                sd = wp.tile([C, CW1], f32, tag="ln_sd", bufs=2)
                nc.scalar.activation(sd[:], var[:], AF.Sqrt,
                                     bias=eps_tile[:, 0:1])
                isdf = wp.tile([C, CW1], f32, tag="ln_isdf", bufs=2)
                nc.vector.reciprocal_approx_fast(out=isdf[:], in_=sd[:])
                return s1[0:C, 0:CW1], isdf

            def ln1_chunk(c):
                sl = slice(c * CW1, (c + 1) * CW1)
                xc = xs[:, sl]
                xbc = wp.tile([C, CW1], bf16, tag="x2b", bufs=2)
                nc.scalar.copy(xbc[:], xc)
                u, isd = ln_stats(xc, xbc[:], eps6)
                xm = wp.tile([C, CW1], f32, tag="ln_xm", bufs=2)
                nc.vector.tensor_tensor(xm[:], xc, u, OP.subtract)
                xn = wp.tile([C, CW1], f32, tag="ln_xn", bufs=2)
                nc.vector.tensor_tensor(xn[:], xm[:], isd[:], OP.mult)
                rl = wp.tile([C, CW1], bf16, tag="ln_rl", bufs=2)
                nc.scalar.activation(rl[:], xn[:], AF.Relu,
                                     bias=ln1b[:, 0:1], scale=ln1w[:, 0:1])
                rsf = colsum96(rl[:])
                # xin_s = S * rl * recip(colsum): scale rides the DVE STT
                lx3 = xin_s[:].rearrange("p (h w) -> p h w", w=WP)
                nc.vector.scalar_tensor_tensor(
                    lx3[0:C, 1 + 8 * c:1 + 8 * c + 8, 1:1 + W],
                    rsf[:].rearrange("p (h w) -> p h w", w=W), S,
                    rl[:].rearrange("p (h w) -> p h w", w=W),
                    OP.mult, OP.mult)

            # ================= MLP epilogue (448px compact chunks) ==========
            hf3 = hbf[:].rearrange("p (h w) -> p h w", w=WP)

            def mlp_p1(c):
                sl = slice(c * CW1, (c + 1) * CW1)
                nc.gpsimd.tensor_tensor(
                    x2s[:, sl].rearrange("p (h w) -> p h w", w=W),
                    xs[:, sl].rearrange("p (h w) -> p h w", w=W),
                    hf3[0:C, 1 + 8 * c:1 + 8 * c + 8, 1:1 + W], OP.add)
                xc = x2s[:, sl]
                x2b = wp.tile([C, CW1], bf16, tag="x2b", bufs=2)
                nc.scalar.copy(x2b[:], xc)
                return ln_stats(xc, x2b[:], eps5)

            def mlp_p2(c, st):
                u, isd = st
                sl = slice(c * CW1, (c + 1) * CW1)
                xc = x2s[:, sl]
                xm = wp.tile([C, CW1], f32, tag="ln_xm", bufs=2)
                nc.vector.tensor_tensor(xm[:], xc, u, OP.subtract)
                xn = wp.tile([C, CW1], bf16, tag="ln_xw", bufs=8)
                nc.vector.tensor_tensor(xn[:], xm[:], isd[:], OP.mult)
                return xn

            def mlp_p3(c, xn):
                ys = []
                for j in range(3):
                    p1 = psum_tile("p1")
                    nc.tensor.matmul(p1[0:128, 0:CW1],
                                     w1s[:, j * 128:(j + 1) * 128], xn[:])
                    y1 = wp.tile([128, CW1], bf16, tag=f"mlp_y{j}",
                                 name=f"mlp_y{j}", bufs=2)
                    if gelu_mode == "hw":
                        nc.scalar.activation(y1[:], p1[0:128, 0:CW1], AF.Gelu,
                                             bias=b1s[:, j:j + 1])
                    else:
                        pre = wp.tile([128, CW1], f32, tag=f"mlp_p{j}",
                                      name=f"mlp_p{j}", bufs=2)
                        nc.scalar.activation(pre[:], p1[0:128, 0:CW1],
                                             AF.Identity, bias=b1s[:, j:j + 1])
                        sg = wp.tile([128, CW1], f32, tag=f"mlp_s{j}",
                                     name=f"mlp_s{j}", bufs=2)
                        nc.scalar.activation(sg[:], pre[:], AF.Sigmoid,
                                             scale=1.702)
                        nc.vector.tensor_tensor(y1[:], pre[:], sg[:], OP.mult)
                    ys.append(y1)
                return ys

            def mlp_p4(c, ys):
                sl = slice(c * CW1, (c + 1) * CW1)
                p2 = psum_tile("p2")
                for k in range(3):
                    nc.tensor.matmul(p2[0:C, 0:CW1], w2s[k][:], ys[k][:],
                                     start=(k == 0), stop=(k == 2))
                oc = wp.tile([C, CW1], f32, tag="oc", bufs=2)
                nc.vector.scalar_tensor_tensor(
                    oc[:], p2[0:C, 0:CW1], b2s[:, 0:1], x2s[:, sl],
                    OP.add, OP.add)
                nc.sync.dma_start(out_d[:, sl], oc[:])

            # ================= the global pipeline =================
            # slot s covers chunk s%7 of iteration s//7.  Lags match the
            # proven baseline: B at 2, norm at 3, b3 at 4.
            total = nit * NCH
            ht_live = {}
            sn_live = {}
            sts = {}
            xns = {}
            yss = {}
            for s in range(0, total + 20):
                if s < total:
                    it0, cc0 = divmod(s, NCH)
                    if it0 == 0:
                        for lc in (2 * cc0, 2 * cc0 + 1):
                            if lc < 7:
                                ln1_chunk(lc)
                    phase_a(it0, cc0)
                c1 = s - 2
                if 0 <= c1 < total:
                    it1, cc1 = divmod(c1, NCH)
                    ht_live[c1] = phase_b(it1, cc1)
                c2 = s - 3
                if 0 <= c2 < total:
                    it2, cc2 = divmod(c2, NCH)
                    sn_live[c2] = phase_n(it2, cc2, ht_live[c2])
                c3 = s - 4
                if 0 <= c3 < total:
                    it3, cc3 = divmod(c3, NCH)
                    phase_b3(it3, cc3, ht_live.pop(c3), sn_live.pop(c3))
                # MLP stages ride the drain: chunk c ready after its final b3
                m1 = s - (total - 7) - 5
                if 0 <= m1 < 7:
                    sts[m1] = mlp_p1(m1)
                m2 = s - (total - 7) - 6
                if 0 <= m2 < 7:
                    xns[m2] = mlp_p2(m2, sts.pop(m2))
                m3 = s - (total - 7) - 7
                if 0 <= m3 < 7:
                    yss[m3] = mlp_p3(m3, xns.pop(m3))
                m4 = s - (total - 7) - 8
                if 0 <= m4 < 7:
                    mlp_p4(m4, yss.pop(m4))

    return nc


def _prepare_maps(x, ln1_w, ln1_b, w_nnmf, ln2_w, ln2_b, w1, b1, w2, b2):
    import ml_dtypes
    bf16 = ml_dtypes.bfloat16
    f8 = ml_dtypes.float8_e4m3
    # eps_eff sized so ratio = xin/(recon+eps_eff) <= ~212 < fp8 max 240:
    # the Exp->fp8 cast produces inf (not saturation) on overflow.
    xf = np.asarray(x, np.float64)
    u = xf.mean(axis=1, keepdims=True)
    v = ((xf - u) ** 2).mean(axis=1, keepdims=True)
    xnorm = ((xf - u) / np.sqrt(v + 1e-6)
             * np.asarray(ln1_w, np.float64)[None, :, None, None]
             + np.asarray(ln1_b, np.float64)[None, :, None, None])
    xr = np.maximum(xnorm, 0.0)
    xin_max = float((xr / (xr.sum(axis=1, keepdims=True) + EPS)).max())
    e_slot = S * max(xin_max, 1e-3) / 200.0
    wrp, wcp, rec0 = _build_weights(w_nnmf, e_slot)
    f = lambda a: np.ascontiguousarray(np.asarray(a, np.float32))
    fb = lambda a: np.ascontiguousarray(np.asarray(a, np.float32).astype(bf16))
    f8c = lambda a: np.ascontiguousarray(np.asarray(a, np.float32).astype(f8))
    w1_64 = np.asarray(w1, np.float64)
    w1f = w1_64 * np.asarray(ln2_w, np.float64)[:, None]
    b1f = np.asarray(b1, np.float64) + np.asarray(ln2_b, np.float64) @ w1_64
    shared = {
        "rec0": fb(rec0),
        "wrp": f8c(wrp),
        "wcp": f8c(wcp),
        "w1T": fb(w1f),
        "b1": f(b1f).reshape(HID, 1),
        "w2T": fb(w2),
        "b2": f(b2).reshape(C, 1),
        "ln1w": f(ln1_w).reshape(C, 1),
        "ln1b": f(ln1_b).reshape(C, 1),
    }
    xs = np.asarray(x)
    return [dict(shared, x=f(xs[i]).reshape(C, NPIX))
            for i in range(xs.shape[0])]


def kernel(x, ln1_w, ln1_b, w_nnmf, ln2_w, ln2_b, w1, b1, w2, b2):
    global _CACHED_NC, LAST_RESULT
    from concourse.bass_utils import run_bass_kernel_spmd

    if _CACHED_NC is None:
        nc = _build_bass()
        nc.finalize()
        _CACHED_NC = nc
    nc = _CACHED_NC
    in_maps = _prepare_maps(x, ln1_w, ln1_b, w_nnmf, ln2_w, ln2_b, w1, b1, w2, b2)
    res = run_bass_kernel_spmd(nc, in_maps, core_ids=list(range(8)), trace=TRACE)
    LAST_RESULT = res
    out = np.stack([res.results[i]["out"].reshape(C, H, W) for i in range(8)])
    return out.astype(np.float32)
